# revision 22
# baseline (speedup 1.0000x reference)
"""LocalAvgPool2D inference kernel for Trainium2, 8-core data-parallel.

Math (from the reference): for input x[b, h, w, c] with H=W=256, C=64 and
LOCAL_SIZE=(64,64), the output is

    core[b, i, j, c] = (1/4096) * sum_{r=i..i+192, w=j..j+192} x[b, r, w, c]
                       for i, j in [0, 64)
    out[b, y, x, c]  = core[b, clamp(y-96, 0, 63), clamp(x-96, 0, 63), c]

i.e. a 193x193 sliding-window sum (scaled by 1/4096) producing a 64x64 core
that is edge-replicated back to 256x256.

Implementation per core (2 batch samples per NeuronCore), TimelineSim
~205us/core, within ~10% of the DMA roofline (64MB of HBM traffic at
~330GB/s):
  1. DMA each row-half as a [128, 16384] tile, loaded as two w-halves
     (contiguous 32KB/partition each) so cumsums overlap the load.
  2. Per-channel inclusive cumsum along W via DVE tensor_tensor_scan
     (in place, strided free dim, chained across w-halves via `initial`).
  3. Pool-engine copy+subtract turn cumsums into 193-wide windowed row
     sums rw[h, (j, c)] = cs[c, 192+j] - cs[c, j-1] in a separate [128,
     4096] tile (frees the big input tile for the next sample's load).
  4. Vertical reduction over the 193-row window via banded-ones matmuls
     (contraction over the 128-row partition dim) accumulating in PSUM,
     with the two j-halves of the core placed on partition quadrants
     0-63 / 64-127 (disjoint PSUM bank ranges) so the core occupies all
     128 partitions; all A0 matmuls run back-to-back, then all A1.
  5. Scaled PSUM->SBUF copies on the Scalar engine (x 1/4096), quadrant
     chunked so each band's corner row is available early.
  6. Output DMAs write the edge-replicated 256x256x64 sample:
     - top/bottom bands (96 identical rows each) first: the padded corner
       row is column-distributed across 128 partitions (512B each) with
       four small SBUF->SBUF DMAs, then one broadcast-source DMA per band
       writes 6MB at full port width;
     - middle rows 96..159: one 128-partition DMA for the core block and
       two pad DMAs fed from 512B-unit pad tiles (>=512B runs avoid the
       sub-512B DMA rate penalty).
"""

import numpy as np

import concourse.bass as bass
import concourse.mybir as mybir
import concourse.tile as tile
from concourse.bass_utils import run_bass_kernel_spmd
from concourse.vector_clock import ScopedClock


class _TC(tile.TileContext):
    """TileContext that limits every instruction to at most one sem wait:
    this toolchain's walrus codegen rejects instructions carrying more than
    one sync-wait command. Extra waits are hoisted into single-wait NoOp
    instructions on the same engine queue immediately before the original
    instruction (sequencers process their queue in order, so this is
    semantically identical)."""

    def _lower_ordered_insts(self, postordered_blocks):
        for insts in postordered_blocks.values():
            if not isinstance(insts, list):
                continue
            new = []
            for inst in insts:
                si = getattr(inst, "sync_info", None)
                try:
                    waits = list(si.on_wait) if si is not None else []
                except Exception:
                    waits = []
                if len(waits) > 1:
                    for k, w in enumerate(waits[:-1]):
                        nop = mybir.InstNoOp(
                            name=f"{inst.name}-ws{k}", ins=[], outs=[]
                        )
                        nop.engine = inst.engine
                        nop.sync_info = mybir.SyncInfo(on_wait=[w], on_update=[])
                        new.append(nop)
                    inst.sync_info.on_wait = [waits[-1]]
                new.append(inst)
            insts[:] = new
        return super()._lower_ordered_insts(postordered_blocks)

    def _drain_and_barrier(self, tick_clock, wait_clock):
        nc = self.nc
        drain_inst = nc.sync.drain()
        wait_clock.add_sem_waits(
            drain_inst.ins, ScopedClock({None: tick_clock.global_clock})
        )
        si = drain_inst.ins.sync_info
        waits = list(si.on_wait) if si is not None else []
        if len(waits) > 1:
            drain_inst.ins.sync_info.on_wait = waits[:1]
            sems = {s.name: s for s in self.sems.allocated().values()}
            for w in waits[1:]:
                d = nc.sync.drain()
                d._wait_ge(sems[w.ant_name], w.wait_value)
        nc.all_engine_barrier()
        popped = nc._tile_sem_poison_stack.pop()
        assert popped is self._sem_poison
        nc.clear_and_free_semaphores(list(self.sems.allocated().values()))
        nc.all_engine_barrier()

B, H, W, C = 16, 256, 256, 64
NCORES = 8
BPC = B // NCORES  # samples per core
KH = KW = 64
PAD = (H - KH) // 2  # 96
SCALE = 1.0 / float(KH * KW)
F32 = mybir.dt.float32
WC = W * C  # 16384 flattened (w, c) row size
JC = KW * C  # 4096 flattened (j, c) core row size
TAIL = WC - JC  # 12288: rw block offset inside the tile


def _body(tc, x, ab, o):
    nc = tc.nc
    add = mybir.AluOpType.add
    bypass = mybir.AluOpType.bypass

    with (
        tc.tile_pool(name="consts", bufs=1) as cpool,
        tc.tile_pool(name="txp", bufs=2) as txpool,
        tc.tile_pool(name="rwp", bufs=2) as rwpool,
        tc.tile_pool(name="outp", bufs=2) as opool,
        tc.tile_pool(name="ftp", bufs=4) as ftpool,
        tc.tile_pool(name="padp", bufs=4) as padpool,
        tc.tile_pool(name="psp", bufs=1, space="PSUM") as pspool,
    ):
        abt = cpool.tile([128, 128], F32, name="abt")
        nc.sync.dma_start(out=abt[:, :], in_=ab[:, :])

        for b in range(BPC):
            txs = []
            for half in range(2):
                t = txpool.tile([128, WC], F32, name=f"t{b}{half}", tag="tx")
                # load in two w-halves so the first half's cumsums can run
                # while the second half is still in flight
                hw_ = WC // 2  # 8192 elems: w < 128
                nc.sync.dma_start(
                    out=t[:, 0:hw_],
                    in_=x[b, half * 128 : (half + 1) * 128, 0 : W // 2].rearrange(
                        "h w c -> h (w c)"
                    ),
                )
                nc.sync.dma_start(
                    out=t[:, hw_:WC],
                    in_=x[b, half * 128 : (half + 1) * 128, W // 2 : W].rearrange(
                        "h w c -> h (w c)"
                    ),
                )
                # per-channel cumsum along w, in place ([128, 128] strided,
                # chained across the two w-halves via `initial`).
                # TensorScalarPtr (scan) is a DVE-only opcode on this ISA.
                eng = nc.vector
                tv = t.rearrange("p (w c) -> p c w", c=C)
                for c in range(C):
                    ch = tv[:, c]
                    if len(ch.shape) == 3:
                        ch = ch.squeeze(1)
                    eng.tensor_tensor_scan(
                        out=ch[:, 0 : W // 2],
                        data0=ch[:, 0 : W // 2],
                        data1=ch[:, 0 : W // 2],
                        initial=0.0,
                        op0=add,
                        op1=bypass,
                    )
                for c in range(C):
                    ch = tv[:, c]
                    if len(ch.shape) == 3:
                        ch = ch.squeeze(1)
                    eng.tensor_tensor_scan(
                        out=ch[:, W // 2 : W],
                        data0=ch[:, W // 2 : W],
                        data1=ch[:, W // 2 : W],
                        initial=t[:, hw_ - C + c : hw_ - C + c + 1],
                        op0=add,
                        op1=bypass,
                    )
                # windowed sums into a separate tile (so the big input tile
                # is released as soon as the subtract finishes, not held
                # through the matmuls): rw[j,c] = cs[c,192+j] - cs[c,j-1],
                # with rw[0,c] = cs[c,192] a plain copy. Both operands are
                # contiguous slices of the scanned tile.
                rw = rwpool.tile([128, JC], F32, name=f"rw{b}{half}", tag="rw")
                nc.gpsimd.tensor_copy(out=rw[:, 0:C], in_=t[:, TAIL : TAIL + C])
                nc.gpsimd.tensor_sub(
                    out=rw[:, C:JC],
                    in0=t[:, TAIL + C : WC],
                    in1=t[:, 0 : 63 * C],
                )
                txs.append(rw)

            # vertical 193-row windowed sums via banded-ones matmuls, split
            # into j-halves across the partition quadrants: PS2[i, q] =
            # core(i, jq) for j < 32, PS2[64+i, q] = core(i, jq) for j >= 32.
            # All output partitions (128) are then live for the output DMAs.
            # All A0 matmuls back-to-back (one weight load, dense PE burst),
            # then all A1 — also releases txs[0]'s pool slot earlier.
            HJ = JC // 2  # 2048: one j-half of a core row
            # PSUM banks are shared across all partitions, so the two
            # partition-quadrants use disjoint column (bank) ranges.
            ps = pspool.tile([128, JC], F32, name=f"ps{b}", tag="ps")
            for lhs, t_, start, stop in (
                (abt[:, 0:64], txs[0], True, False),
                (abt[:, 64:128], txs[1], False, True),
            ):
                for hx in range(2):
                    for fc in range(4):
                        sl = slice(hx * HJ + fc * 512, hx * HJ + (fc + 1) * 512)
                        nc.tensor.matmul(
                            ps[hx * 64 : (hx + 1) * 64, sl],
                            lhs,
                            t_[:, sl],
                            start=start,
                            stop=stop,
                        )

            # scaled PSUM->SBUF in quadrant chunks; the top band depends only
            # on chunks [0:32] + [64:96] (row 0), the bottom on the rest.
            outsb = opool.tile([128, HJ], F32, name=f"osb{b}", tag="osb")
            for q0, q1 in ((0, 32), (64, 96), (32, 64), (96, 128)):
                hx = q0 // 64
                nc.scalar.activation(
                    outsb[q0:q1, :],
                    ps[q0:q1, hx * HJ : (hx + 1) * HJ],
                    mybir.ActivationFunctionType.Copy,
                    bias=0.0,
                    scale=SCALE,
                )

            ob = o[b]
            # top/bottom bands first: 96 identical rows each, depending only
            # on corner row 0 resp. 63. Build the padded corner row
            # column-distributed across 128 partitions (512B per partition)
            # with four small SBUF->SBUF DMAs, then write the whole band
            # with one broadcast-source DMA at full port width.
            for row_i, (y0, y1) in ((0, (0, PAD)), (63, (PAD + KH, H))):
                ft = ftpool.tile([128, 128], F32, name=f"ft{b}{row_i}", tag="ft")
                rl = outsb[row_i : row_i + 1]  # j-half 0 of core row
                rr = outsb[64 + row_i : 65 + row_i]  # j-half 1
                nc.gpsimd.dma_start(
                    out=ft[0:48, :],
                    in_=rl[:, 0:C].unsqueeze(1).broadcast_to([1, 2 * 48, C]),
                )
                nc.gpsimd.dma_start(out=ft[48:64, :], in_=rl[:, :])
                nc.gpsimd.dma_start(out=ft[64:80, :], in_=rr[:, :])
                nc.gpsimd.dma_start(
                    out=ft[80:128, :],
                    in_=rr[:, HJ - C : HJ].unsqueeze(1).broadcast_to([1, 2 * 48, C]),
                )
                n = y1 - y0
                nc.scalar.dma_start(
                    out=ob[y0:y1].rearrange("y (xh xl) c -> xh y (xl c)", xl=2),
                    in_=ft.unsqueeze(1).broadcast_to([128, n, 128]),
                )
            # 512B pad units (corner value duplicated x2, per row) so the
            # x-edge replication DMAs read >=512B contiguous runs. padl uses
            # partitions 0..63 (where the j=0 corners live), padr partitions
            # 64..127 (where the j=63 corners live) — engine-local copies.
            pad = padpool.tile([128, 2 * C], F32, name=f"pad{b}", tag="pad")
            nc.vector.tensor_copy(
                out=pad[0:64].rearrange("p (r c) -> p r c", c=C),
                in_=outsb[0:64, 0:C].unsqueeze(1).broadcast_to([64, 2, C]),
            )
            nc.vector.tensor_copy(
                out=pad[64:128].rearrange("p (r c) -> p r c", c=C),
                in_=outsb[64:128, HJ - C : HJ].unsqueeze(1).broadcast_to([64, 2, C]),
            )
            # middle band rows 96..159: core write uses all 128 partitions.
            nc.scalar.dma_start(
                out=ob[PAD : PAD + KH, PAD : PAD + KW, :].rearrange(
                    "y (hx xr) c -> hx y (xr c)", hx=2
                ),
                in_=outsb[:, :],
            )
            nc.scalar.dma_start(
                out=ob[PAD : PAD + KH, 0:PAD, :],
                in_=pad[0:64].unsqueeze(1).broadcast_to([64, PAD // 2, 2 * C]),
            )
            nc.scalar.dma_start(
                out=ob[PAD : PAD + KH, PAD + KW : W, :],
                in_=pad[64:128].unsqueeze(1).broadcast_to([64, PAD // 2, 2 * C]),
            )


def _build():
    nc = bass.Bass("TRN2")
    x = nc.declare_dram_parameter("x", [BPC, H, W, C], F32, isOutput=False)
    ab = nc.declare_dram_parameter("ab", [128, 128], F32, isOutput=False)
    o = nc.declare_dram_parameter("o", [BPC, H, W, C], F32, isOutput=True)
    with _TC(nc) as tc:
        _body(tc, x, ab, o)
    return nc


def _ab_const() -> np.ndarray:
    # A0[h, i] = 1 if h >= i           (rows 0..127 of the 193-row window)
    # A1[h, i] = 1 if h <= i + 64      (rows 128..255)
    ab = np.zeros((128, 128), np.float32)
    h = np.arange(128)[:, None]
    i = np.arange(64)[None, :]
    ab[:, 0:64] = (h >= i).astype(np.float32)
    ab[:, 64:128] = (h <= i + 64).astype(np.float32)
    return ab


def _run(x: np.ndarray, trace: bool = False, **kwargs):
    nc = _build()
    ab = _ab_const()
    in_maps = [
        {"x": x[ci * BPC : (ci + 1) * BPC], "ab": ab} for ci in range(NCORES)
    ]
    res = run_bass_kernel_spmd(
        nc, in_maps, list(range(NCORES)), trace=trace, **kwargs
    )
    out = np.concatenate([res.results[ci]["o"] for ci in range(NCORES)], axis=0)
    return out, res


def kernel(inputs: np.ndarray) -> np.ndarray:
    x = np.ascontiguousarray(np.asarray(inputs, dtype=np.float32))
    assert x.shape == (B, H, W, C), x.shape
    out, _ = _run(x, trace=False)
    return out


if __name__ == "__main__":
    rng = np.random.default_rng(0)
    x = rng.standard_normal((B, H, W, C), dtype=np.float32)
    y = kernel(x)
    print(y.shape, y.dtype)


# revision 31
# speedup vs baseline: 1.0368x; 1.0368x over previous
"""LocalAvgPool2D inference kernel for Trainium2, 8-core data-parallel.

Math (from the reference): for input x[b, h, w, c] with H=W=256, C=64 and
LOCAL_SIZE=(64,64), the output is

    core[b, i, j, c] = (1/4096) * sum_{r=i..i+192, w=j..j+192} x[b, r, w, c]
                       for i, j in [0, 64)
    out[b, y, x, c]  = core[b, clamp(y-96, 0, 63), clamp(x-96, 0, 63), c]

i.e. a 193x193 sliding-window sum (scaled by 1/4096) producing a 64x64 core
that is edge-replicated back to 256x256.

Implementation per core (2 batch samples per NeuronCore), TimelineSim
~205us/core, within ~10% of the DMA roofline (64MB of HBM traffic at
~330GB/s):
  1. DMA each row-half as a [128, 16384] tile, loaded as two w-halves
     (contiguous 32KB/partition each) so cumsums overlap the load.
  2. Per-channel inclusive cumsum along W via DVE tensor_tensor_scan
     (in place, strided free dim, chained across w-halves via `initial`).
  3. Pool-engine copy+subtract turn cumsums into 193-wide windowed row
     sums rw[h, (j, c)] = cs[c, 192+j] - cs[c, j-1] in a separate [128,
     4096] tile (frees the big input tile for the next sample's load).
  4. Vertical reduction over the 193-row window via banded-ones matmuls
     (contraction over the 128-row partition dim) accumulating in PSUM,
     with the two j-halves of the core placed on partition quadrants
     0-63 / 64-127 (disjoint PSUM bank ranges) so the core occupies all
     128 partitions; all A0 matmuls run back-to-back, then all A1.
  5. Scaled PSUM->SBUF copies on the Scalar engine (x 1/4096), quadrant
     chunked so each band's corner row is available early.
  6. Output DMAs write the edge-replicated 256x256x64 sample:
     - top/bottom bands (96 identical rows each) first: the padded corner
       row is column-distributed across 128 partitions (512B each) with
       four small SBUF->SBUF DMAs, then one broadcast-source DMA per band
       writes 6MB at full port width;
     - middle rows 96..159: one 128-partition DMA for the core block and
       two pad DMAs fed from 512B-unit pad tiles (>=512B runs avoid the
       sub-512B DMA rate penalty).
"""

import numpy as np

import concourse.bass as bass
import concourse.mybir as mybir
import concourse.tile as tile
from concourse.bass_utils import run_bass_kernel_spmd
from concourse.vector_clock import ScopedClock


class _TC(tile.TileContext):
    """TileContext that limits every instruction to at most one sem wait:
    this toolchain's walrus codegen rejects instructions carrying more than
    one sync-wait command. Extra waits are hoisted into single-wait NoOp
    instructions on the same engine queue immediately before the original
    instruction (sequencers process their queue in order, so this is
    semantically identical)."""

    def _lower_ordered_insts(self, postordered_blocks):
        for insts in postordered_blocks.values():
            if not isinstance(insts, list):
                continue
            new = []
            for inst in insts:
                si = getattr(inst, "sync_info", None)
                try:
                    waits = list(si.on_wait) if si is not None else []
                except Exception:
                    waits = []
                if len(waits) > 1:
                    for k, w in enumerate(waits[:-1]):
                        nop = mybir.InstNoOp(
                            name=f"{inst.name}-ws{k}", ins=[], outs=[]
                        )
                        nop.engine = inst.engine
                        nop.sync_info = mybir.SyncInfo(on_wait=[w], on_update=[])
                        new.append(nop)
                    inst.sync_info.on_wait = [waits[-1]]
                new.append(inst)
            insts[:] = new
        return super()._lower_ordered_insts(postordered_blocks)

    def _drain_and_barrier(self, tick_clock, wait_clock):
        nc = self.nc
        drain_inst = nc.sync.drain()
        wait_clock.add_sem_waits(
            drain_inst.ins, ScopedClock({None: tick_clock.global_clock})
        )
        si = drain_inst.ins.sync_info
        waits = list(si.on_wait) if si is not None else []
        if len(waits) > 1:
            drain_inst.ins.sync_info.on_wait = waits[:1]
            sems = {s.name: s for s in self.sems.allocated().values()}
            for w in waits[1:]:
                d = nc.sync.drain()
                d._wait_ge(sems[w.ant_name], w.wait_value)
        nc.all_engine_barrier()
        popped = nc._tile_sem_poison_stack.pop()
        assert popped is self._sem_poison
        nc.clear_and_free_semaphores(list(self.sems.allocated().values()))
        nc.all_engine_barrier()

B, H, W, C = 16, 256, 256, 64
NCORES = 8
BPC = B // NCORES  # samples per core
KH = KW = 64
PAD = (H - KH) // 2  # 96
SCALE = 1.0 / float(KH * KW)
F32 = mybir.dt.float32
WC = W * C  # 16384 flattened (w, c) row size
JC = KW * C  # 4096 flattened (j, c) core row size
TAIL = WC - JC  # 12288: rw block offset inside the tile


def _body(tc, x, ab, o):
    nc = tc.nc
    add = mybir.AluOpType.add
    bypass = mybir.AluOpType.bypass

    with (
        tc.tile_pool(name="consts", bufs=1) as cpool,
        tc.tile_pool(name="txp", bufs=2) as txpool,
        tc.tile_pool(name="rwp", bufs=5) as rwpool,
        tc.tile_pool(name="outp", bufs=2) as opool,
        tc.tile_pool(name="ftp", bufs=8) as ftpool,
        tc.tile_pool(name="padp", bufs=8) as padpool,
        tc.tile_pool(name="psp", bufs=1, space="PSUM") as pspool,
    ):
        abt = cpool.tile([128, 128], F32, name="abt")
        nc.gpsimd.dma_start(out=abt[:, :], in_=ab[:, :])

        for b in range(BPC):
            txs = []
            for half in range(2):
                t = txpool.tile([128, WC], F32, name=f"t{b}{half}", tag="tx")
                # load in two w-halves so the first half's cumsums can run
                # while the second half is still in flight
                hw_ = WC // 2  # 8192 elems: w < 128
                nc.sync.dma_start(
                    out=t[:, 0:hw_],
                    in_=x[b, half * 128 : (half + 1) * 128, 0 : W // 2].rearrange(
                        "h w c -> h (w c)"
                    ),
                )
                nc.sync.dma_start(
                    out=t[:, hw_:WC],
                    in_=x[b, half * 128 : (half + 1) * 128, W // 2 : W].rearrange(
                        "h w c -> h (w c)"
                    ),
                )
                # per-channel cumsum along w, in place ([128, 128] strided,
                # chained across the two w-halves via `initial`).
                # TensorScalarPtr (scan) is a DVE-only opcode on this ISA.
                eng = nc.vector
                tv = t.rearrange("p (w c) -> p c w", c=C)
                for c in range(C):
                    ch = tv[:, c]
                    if len(ch.shape) == 3:
                        ch = ch.squeeze(1)
                    eng.tensor_tensor_scan(
                        out=ch[:, 0 : W // 2],
                        data0=ch[:, 0 : W // 2],
                        data1=ch[:, 0 : W // 2],
                        initial=0.0,
                        op0=add,
                        op1=bypass,
                    )
                for c in range(C):
                    ch = tv[:, c]
                    if len(ch.shape) == 3:
                        ch = ch.squeeze(1)
                    eng.tensor_tensor_scan(
                        out=ch[:, W // 2 : W],
                        data0=ch[:, W // 2 : W],
                        data1=ch[:, W // 2 : W],
                        initial=t[:, hw_ - C + c : hw_ - C + c + 1],
                        op0=add,
                        op1=bypass,
                    )
                # windowed sums into a separate tile (so the big input tile
                # is released as soon as the subtract finishes, not held
                # through the matmuls): rw[j,c] = cs[c,192+j] - cs[c,j-1],
                # with rw[0,c] = cs[c,192] a plain copy. Both operands are
                # contiguous slices of the scanned tile.
                HJ_ = JC // 2
                rw0 = rwpool.tile([128, HJ_], F32, name=f"rw{b}{half}0", tag="rwh")
                rw1 = rwpool.tile([128, HJ_], F32, name=f"rw{b}{half}1", tag="rwh")
                nc.gpsimd.tensor_copy(out=rw0[:, 0:C], in_=t[:, TAIL : TAIL + C])
                nc.gpsimd.tensor_sub(
                    out=rw0[:, C:HJ_],
                    in0=t[:, TAIL + C : TAIL + HJ_],
                    in1=t[:, 0 : HJ_ - C],
                )
                nc.gpsimd.tensor_sub(
                    out=rw1[:, :],
                    in0=t[:, TAIL + HJ_ : WC],
                    in1=t[:, HJ_ - C : JC - C],
                )
                txs.append((rw0, rw1))

            # vertical 193-row windowed sums via banded-ones matmuls, split
            # into j-halves across the partition quadrants: PS2[i, q] =
            # core(i, jq) for j < 32, PS2[64+i, q] = core(i, jq) for j >= 32.
            # All output partitions (128) are then live for the output DMAs.
            # All A0 matmuls back-to-back (one weight load, dense PE burst),
            # then all A1 — also releases txs[0]'s pool slot earlier.
            HJ = JC // 2  # 2048: one j-half of a core row
            # PSUM banks are shared across all partitions, so the two
            # partition-quadrants use disjoint column (bank) ranges.
            ps = pspool.tile([128, JC], F32, name=f"ps{b}", tag="ps")
            for lhs, t_, start, stop in (
                (abt[:, 0:64], txs[0], True, False),
                (abt[:, 64:128], txs[1], False, True),
            ):
                for hx in range(2):
                    for fc in range(4):
                        sl = slice(hx * HJ + fc * 512, hx * HJ + (fc + 1) * 512)
                        nc.tensor.matmul(
                            ps[hx * 64 : (hx + 1) * 64, sl],
                            lhs,
                            t_[hx][:, fc * 512 : (fc + 1) * 512],
                            start=start,
                            stop=stop,
                        )

            # scaled PSUM->SBUF: the two quadrants drain on different
            # engines (ACT + DVE) so they run concurrently and the band
            # corner rows become available after one copy latency.
            outsb = opool.tile([128, HJ], F32, name=f"osb{b}", tag="osb")
            nc.scalar.activation(
                outsb[0:64, :],
                ps[0:64, 0:HJ],
                mybir.ActivationFunctionType.Copy,
                bias=0.0,
                scale=SCALE,
            )
            nc.vector.tensor_scalar_mul(outsb[64:128, :], ps[64:128, HJ:JC], SCALE)

            ob = o[b]
            # top/bottom bands first: 96 identical rows each, depending only
            # on corner row 0 resp. 63. Build the padded corner row
            # column-distributed across 128 partitions (512B per partition)
            # with four small SBUF->SBUF DMAs, then write the whole band
            # with one broadcast-source DMA at full port width.
            for row_i, (y0, y1) in ((0, (0, PAD)), (63, (PAD + KH, H))):
                ft = ftpool.tile([128, 128], F32, name=f"ft{b}{row_i}", tag="ft")
                rl = outsb[row_i : row_i + 1]  # j-half 0 of core row
                rr = outsb[64 + row_i : 65 + row_i]  # j-half 1
                nc.gpsimd.dma_start(
                    out=ft[0:48, :],
                    in_=rl[:, 0:C].unsqueeze(1).broadcast_to([1, 2 * 48, C]),
                )
                nc.gpsimd.dma_start(out=ft[48:64, :], in_=rl[:, :])
                nc.gpsimd.dma_start(out=ft[64:80, :], in_=rr[:, :])
                nc.gpsimd.dma_start(
                    out=ft[80:128, :],
                    in_=rr[:, HJ - C : HJ].unsqueeze(1).broadcast_to([1, 2 * 48, C]),
                )
                n = y1 - y0
                nc.scalar.dma_start(
                    out=ob[y0:y1].rearrange("y (xh xl) c -> xh y (xl c)", xl=2),
                    in_=ft.unsqueeze(1).broadcast_to([128, n, 128]),
                )
            # 512B pad units (corner value duplicated x2, per row) so the
            # x-edge replication DMAs read >=512B contiguous runs. padl uses
            # partitions 0..63 (where the j=0 corners live), padr partitions
            # 64..127 (where the j=63 corners live) — engine-local copies.
            pad = padpool.tile([128, 2 * C], F32, name=f"pad{b}", tag="pad")
            nc.vector.tensor_copy(
                out=pad[0:64].rearrange("p (r c) -> p r c", c=C),
                in_=outsb[0:64, 0:C].unsqueeze(1).broadcast_to([64, 2, C]),
            )
            nc.vector.tensor_copy(
                out=pad[64:128].rearrange("p (r c) -> p r c", c=C),
                in_=outsb[64:128, HJ - C : HJ].unsqueeze(1).broadcast_to([64, 2, C]),
            )
            # middle band rows 96..159: core write uses all 128 partitions.
            nc.scalar.dma_start(
                out=ob[PAD : PAD + KH, PAD : PAD + KW, :].rearrange(
                    "y (hx xr) c -> hx y (xr c)", hx=2
                ),
                in_=outsb[:, :],
            )
            nc.scalar.dma_start(
                out=ob[PAD : PAD + KH, 0:PAD, :],
                in_=pad[0:64].unsqueeze(1).broadcast_to([64, PAD // 2, 2 * C]),
            )
            nc.scalar.dma_start(
                out=ob[PAD : PAD + KH, PAD + KW : W, :],
                in_=pad[64:128].unsqueeze(1).broadcast_to([64, PAD // 2, 2 * C]),
            )


def _build():
    nc = bass.Bass("TRN2")
    x = nc.declare_dram_parameter("x", [BPC, H, W, C], F32, isOutput=False)
    ab = nc.declare_dram_parameter("ab", [128, 128], F32, isOutput=False)
    o = nc.declare_dram_parameter("o", [BPC, H, W, C], F32, isOutput=True)
    with _TC(nc) as tc:
        _body(tc, x, ab, o)
    return nc


def _ab_const() -> np.ndarray:
    # A0[h, i] = 1 if h >= i           (rows 0..127 of the 193-row window)
    # A1[h, i] = 1 if h <= i + 64      (rows 128..255)
    ab = np.zeros((128, 128), np.float32)
    h = np.arange(128)[:, None]
    i = np.arange(64)[None, :]
    ab[:, 0:64] = (h >= i).astype(np.float32)
    ab[:, 64:128] = (h <= i + 64).astype(np.float32)
    return ab


def _run(x: np.ndarray, trace: bool = False, **kwargs):
    nc = _build()
    ab = _ab_const()
    in_maps = [
        {"x": x[ci * BPC : (ci + 1) * BPC], "ab": ab} for ci in range(NCORES)
    ]
    res = run_bass_kernel_spmd(
        nc, in_maps, list(range(NCORES)), trace=trace, **kwargs
    )
    out = np.concatenate([res.results[ci]["o"] for ci in range(NCORES)], axis=0)
    return out, res


def kernel(inputs: np.ndarray) -> np.ndarray:
    x = np.ascontiguousarray(np.asarray(inputs, dtype=np.float32))
    assert x.shape == (B, H, W, C), x.shape
    out, _ = _run(x, trace=False)
    return out


if __name__ == "__main__":
    rng = np.random.default_rng(0)
    x = rng.standard_normal((B, H, W, C), dtype=np.float32)
    y = kernel(x)
    print(y.shape, y.dtype)
